# revision 33
# baseline (speedup 1.0000x reference)
"""Trainium2 Bass kernel for nn_Attention (dense transformer block, full-dim attention).

Reference computation (per batch b):
    qn/kn/vn = LayerNorm(q/k/v[b])           # over C=256
    qp = qn @ Wq + bq                        # [N, INNER]
    kp = kn @ Wk + bk
    vp = vn @ Wv + bv
    S  = qp @ kp.T * INNER_HEAD^-0.5         # [N, N]
    P  = softmax(S, axis=-1)
    out= (P @ vp) @ Wo + bo                  # [N, C]

Sharding: 8 cores = 4 batches x 2 query-row halves. Each core handles one
batch's full k/v (duplicated LN+projection within the pair -- no collectives)
and 1024 of the 2048 query rows.

On-chip dataflow (per core), all matmul contractions on the partition dim:
    xnT      : LN in natural layout ([tok,C] tiles), DMA-transpose to [C, tok]
    qpT, kpT : [INNER, tok] = Wq/Wk chunks (lhsT) x xnT (rhs)
    vp       : [tok, INNER] = vnT tiles (lhsT) x Wv (rhs)
    S^T      : [ktok, qtok] = kpT tiles (lhsT) x qpT (rhs), PSUM f32
    expS^T   : ACT exp(0.125 * S^T) straight out of PSUM, bf16
    rowsum   : ones-vector matmuls over expS^T (accumulated in PSUM)
    X~T      : [INNER, qtok] = vp tiles (lhsT) x expS^T (rhs)  [unnormalized]
    out      : [qtok, C] = X~T tiles (lhsT) x Wo (rhs); epilogue divides by
               rowsum (softmax normalization commutes with the row-linear
               output projection) and adds bo.
"""

import numpy as np
import ml_dtypes

import concourse.bass as bass
import concourse.tile as tile
from concourse import mybir
from concourse.bass_utils import run_bass_kernel_spmd

# Problem shapes (hardcoded per contract)
B = 4
N = 2048          # sequence length (k/v tokens per core)
C = 256           # channels
INNER = 1024      # inner projection dim
NQ = 1024         # query rows per core (N/2)
EPS = 1e-5
SCALE = 0.125     # 64 ** -0.5
P = 128

FP = mybir.dt.float32
BF = mybir.dt.bfloat16

NCORES = 8
CCH = C // P          # 2 chunks of the channel dim
JT = INNER // P       # 8 tiles of the inner dim
MT = N // P           # 16 k-token tiles
QT = NQ // P          # 8 q-token tiles
QCH = NQ // 512       # 2 q-token free chunks
KCH = N // 512        # 4 k-token free chunks

_sub = mybir.AluOpType.subtract
_mult = mybir.AluOpType.mult


def _bcast(ap, parts=P):
    # prepend a stride-0 partition dim: [n] -> [parts, n]
    return bass.AP(tensor=ap.tensor, offset=ap.offset,
                   ap=[[0, parts]] + [list(d) for d in ap.ap])


def _emit(nc, tc, io):
    from contextlib import ExitStack

    with ExitStack() as ctx:
        consts = ctx.enter_context(tc.tile_pool(name="consts", bufs=1))
        big = ctx.enter_context(tc.tile_pool(name="big", bufs=1))
        ln_pool = ctx.enter_context(tc.tile_pool(name="ln", bufs=3))
        lnx_pool = ctx.enter_context(tc.tile_pool(name="lnx", bufs=9))
        stat = ctx.enter_context(tc.tile_pool(name="stat", bufs=4))
        temps = ctx.enter_context(tc.tile_pool(name="temps", bufs=3))
        psum = ctx.enter_context(tc.tile_pool(name="psum", bufs=5, space="PSUM"))
        psum_rs = ctx.enter_context(tc.tile_pool(name="psum_rs", bufs=1, space="PSUM"))
        psum_t = ctx.enter_context(tc.tile_pool(name="psum_t", bufs=2, space="PSUM"))

        # ---- constants ------------------------------------------------
        # gamma/beta live on partitions in the transposed layout: [128, CCH]
        gT_sb = consts.tile([P, CCH], FP)
        nc.sync.dma_start(gT_sb, io["gamma"].rearrange("(c p) -> p c", p=P))
        bT_sb = consts.tile([P, CCH], FP)
        nc.sync.dma_start(bT_sb, io["beta"].rearrange("(c p) -> p c", p=P))
        bo_b = consts.tile([P, C], FP)
        nc.gpsimd.dma_start(bo_b, _bcast(io["bo"]))

        bq_sb = consts.tile([P, JT], FP)
        nc.scalar.dma_start(bq_sb, io["bq"].rearrange("(j p) -> p j", p=P))
        bk_sb = consts.tile([P, JT], FP)
        nc.scalar.dma_start(bk_sb, io["bk"].rearrange("(j p) -> p j", p=P))

        wq_sb = consts.tile([P, CCH, INNER], BF)
        nc.scalar.dma_start(wq_sb, io["Wq"].rearrange("(c p) n -> p c n", p=P))
        wk_sb = consts.tile([P, CCH, INNER], BF)
        nc.scalar.dma_start(wk_sb, io["Wk"].rearrange("(c p) n -> p c n", p=P))
        wv_sb = consts.tile([P, CCH, INNER], BF)
        nc.scalar.dma_start(wv_sb, io["Wv"].rearrange("(c p) n -> p c n", p=P))
        wo_sb = consts.tile([P, JT, C], BF)
        nc.scalar.dma_start(wo_sb, io["Wo"].rearrange("(j p) n -> p j n", p=P))

        ones_sb = consts.tile([P, 1], BF)
        nc.vector.memset(ones_sb, 1.0)
        eps_sb = consts.tile([P, 1], FP)
        nc.vector.memset(eps_sb, EPS)
        ident = consts.tile([P, P], BF)
        from concourse.masks import make_identity
        make_identity(nc, ident)

        # ---- persistent activations ----------------------------------
        qnT = big.tile([P, CCH, NQ], BF)
        knT = big.tile([P, CCH, N], BF)
        vnT = big.tile([P, CCH, N], BF)
        qpT = big.tile([P, JT, NQ], BF)
        kpT = big.tile([P, JT, N], BF)
        vp = big.tile([P, MT, INNER], BF)
        expS = big.tile([P, MT, NQ], BF)
        xT = big.tile([P, JT, NQ], BF)
        recip = big.tile([P, QT], FP)

        # ---- phase 1: layernorm + transpose --------------------------
        def layernorm(x_dram, ntiles, dstT):
            # groups of 8 tiles: batched stats -> one sqrt/recip per group,
            # then apply + transpose via PE (idle during this phase)
            for g0 in range(0, ntiles, 8):
                gn = min(8, ntiles - g0)
                mv_g = stat.tile([P, 8, 2], FP, tag="mv_g")
                xts = []
                for ii in range(gn):
                    i = g0 + ii
                    xt = lnx_pool.tile([P, C], FP, tag="xt")
                    nc.sync.dma_start(xt, x_dram[i * P:(i + 1) * P, :])
                    st = stat.tile([P, 6], FP, tag="st")
                    nc.vector.bn_stats(st, xt)
                    nc.vector.bn_aggr(mv_g[:, ii, :], st)
                    xts.append(xt)
                rstd_g = stat.tile([P, 8], FP, tag="rstd_g")
                nc.scalar.activation(rstd_g[:, :gn], mv_g[:, :gn, 1],
                                     mybir.ActivationFunctionType.Sqrt,
                                     bias=eps_sb, scale=1.0)
                nc.vector.reciprocal(rstd_g[:, :gn], rstd_g[:, :gn])
                for ii in range(gn):
                    i = g0 + ii
                    xn = ln_pool.tile([P, C], BF, tag="xn")
                    nc.vector.tensor_scalar(xn, xts[ii], mv_g[:, ii, 0:1],
                                            rstd_g[:, ii:ii + 1],
                                            op0=_sub, op1=_mult)
                    for c in range(CCH):
                        pst = psum_t.tile([P, P], BF, tag="pst")
                        nc.tensor.transpose(pst, xn[:, c * P:(c + 1) * P],
                                            ident)
                        # gamma/beta fused into the PSUM eviction: in the
                        # transposed layout they are per-partition scalars
                        dst = dstT[:, c, i * P:(i + 1) * P]
                        if (i + c) % 2 == 0:
                            nc.vector.tensor_scalar(dst, pst,
                                                    gT_sb[:, c:c + 1],
                                                    bT_sb[:, c:c + 1],
                                                    op0=_mult,
                                                    op1=mybir.AluOpType.add)
                        else:
                            nc.scalar.activation(
                                dst, pst,
                                mybir.ActivationFunctionType.Identity,
                                bias=bT_sb[:, c:c + 1],
                                scale=gT_sb[:, c:c + 1])

        # ---- phase 2: projections (interleaved per tensor with LN) ---
        def proj_T(srcT, w_sb, b_sb, dst, nch, evict_act):
            # dst[P(inner j), j, tok] = (x @ W).T + b   per inner tile j
            for j in range(JT):
                for n in range(nch):
                    ps = psum.tile([P, 512], FP, tag="ps")
                    for c in range(CCH):
                        nc.tensor.matmul(ps,
                                         lhsT=w_sb[:, c, j * P:(j + 1) * P],
                                         rhs=srcT[:, c, n * 512:(n + 1) * 512],
                                         start=(c == 0), stop=(c == CCH - 1))
                    d = dst[:, j, n * 512:(n + 1) * 512]
                    if evict_act and (j + n) % 2 == 0:
                        nc.scalar.activation(
                            d, ps, mybir.ActivationFunctionType.Identity,
                            bias=b_sb[:, j:j + 1], scale=1.0)
                    else:
                        nc.vector.tensor_scalar_add(d, ps, b_sb[:, j:j + 1])

        layernorm(io["xq"], QT, qnT)
        proj_T(qnT, wq_sb, bq_sb, qpT, QCH, evict_act=False)
        layernorm(io["xk"], MT, knT)
        proj_T(knT, wk_sb, bk_sb, kpT, KCH, evict_act=True)
        layernorm(io["xv"], MT, vnT)

        # vp natural: [tok(m), INNER]
        for m in range(MT):
            for jc in range(2):
                ps = psum.tile([P, 512], FP, tag="ps")
                for c in range(CCH):
                    nc.tensor.matmul(ps,
                                     lhsT=vnT[:, c, m * P:(m + 1) * P],
                                     rhs=wv_sb[:, c, jc * 512:(jc + 1) * 512],
                                     start=(c == 0), stop=(c == CCH - 1))
                d = vp[:, m, jc * 512:(jc + 1) * 512]
                if (m + jc) % 2 == 0:
                    nc.vector.tensor_copy(d, ps)
                else:
                    nc.scalar.copy(d, ps)

        # ---- phase 3: S^T = kp @ qp.T, exp ---------------------------
        for m in range(MT):
            for n in range(QCH):
                ps = psum.tile([P, 512], FP, tag="ps")
                for j in range(JT):
                    nc.tensor.matmul(ps,
                                     lhsT=kpT[:, j, m * P:(m + 1) * P],
                                     rhs=qpT[:, j, n * 512:(n + 1) * 512],
                                     start=(j == 0), stop=(j == JT - 1))
                nc.scalar.activation(expS[:, m, n * 512:(n + 1) * 512], ps,
                                     mybir.ActivationFunctionType.Exp,
                                     scale=SCALE)

        # rowsums: lhsT = expS tile, rhs = ones -> [128,1] per q-tile,
        # all 8 tiles sharing one PSUM bank (sequential accumulation groups)
        rs8 = psum_rs.tile([P, QT], FP, tag="rs8")
        for t in range(QT):
            for m in range(MT):
                nc.tensor.matmul(rs8[:, t:t + 1],
                                 lhsT=expS[:, m, t * P:(t + 1) * P],
                                 rhs=ones_sb,
                                 start=(m == 0), stop=(m == MT - 1))
        nc.vector.reciprocal(recip, rs8)

        # ---- phase 4: X~T = vp.T-tiles x expS^T ----------------------
        for n in range(QCH):
            for j in range(JT):
                ps = psum.tile([P, 512], FP, tag="ps")
                for m in range(MT):
                    nc.tensor.matmul(ps,
                                     lhsT=vp[:, m, j * P:(j + 1) * P],
                                     rhs=expS[:, m, n * 512:(n + 1) * 512],
                                     start=(m == 0), stop=(m == MT - 1))
                nc.vector.tensor_copy(xT[:, j, n * 512:(n + 1) * 512], ps)

        # ---- phase 5: out = X~T.T @ Wo, normalize + bias -------------
        for t in range(QT):
            ps = psum.tile([P, 512], FP, tag="ps")
            po = ps[:, :C]
            for j in range(JT):
                nc.tensor.matmul(po,
                                 lhsT=xT[:, j, t * P:(t + 1) * P],
                                 rhs=wo_sb[:, j, :],
                                 start=(j == 0), stop=(j == JT - 1))
            o1 = temps.tile([P, C], FP, tag="o1")
            nc.vector.tensor_scalar(o1, po, recip[:, t:t + 1], None, op0=_mult)
            o2 = temps.tile([P, C], FP, tag="o2")
            nc.vector.tensor_add(o2, o1, bo_b)
            nc.sync.dma_start(io["out"][t * P:(t + 1) * P, :], o2)


_DMA_WAIT_LIMIT = 1
_ENGINE_WAIT_LIMIT = 1


def _split_dma_waits(nc, wsem):
    """This walrus's instruction structs carry very few sync-wait slots
    (DMA_DIRECT2D effectively 1, engine ops ~2); Tile can emit more. Move the
    excess onto an EventSemaphore wait on the issuing engine right before the
    instruction (engine streams are in-order, so this is a conservative,
    correct strengthening)."""
    import bass_rust
    fn = nc.m.functions[0]
    for blk in fn.blocks:
        il = list(blk.instructions)
        out = []
        changed = False
        for inst in il:
            tn = type(inst).__name__
            si = inst.sync_info
            if si is not None and tn != "InstEventSemaphore":
                limit = _DMA_WAIT_LIMIT if ("DMA" in tn or "Dma" in tn) \
                    else _ENGINE_WAIT_LIMIT
                w = list(si.on_wait)
                if len(w) > limit:
                    excess = w[:-limit]
                    # EventSemaphore carries <=2 waits and <=1 update; chain
                    # as many as needed, each ticking the dummy wsplit sem.
                    for gi in range(0, len(excess), 2):
                        nop = mybir.InstEventSemaphore(
                            name=f"wsplit{gi}_{inst.name}", ins=[], outs=[])
                        nop.engine = inst.engine
                        nop.sync_info = bass_rust.SyncInfo(
                            on_wait=excess[gi:gi + 2],
                            on_update=[bass_rust.SyncUpdate(
                                sync_type="semaphore", id=wsem.num,
                                ant_name=wsem.name, update_mode="sem-add-imm",
                                update_value=1)])
                        out.append(nop)
                    si.on_wait = w[-limit:]
                    changed = True
            out.append(inst)
        if changed:
            blk.instructions = out


_NC_CACHE = None


def build_nc():
    global _NC_CACHE
    if _NC_CACHE is not None:
        return _NC_CACHE
    nc = bass.Bass("TRN2", target_bir_lowering=False, debug=False,
                   num_devices=NCORES)
    io = {}
    io["xq"] = nc.dram_tensor("xq", [NQ, C], FP, kind="ExternalInput").ap()
    io["xk"] = nc.dram_tensor("xk", [N, C], FP, kind="ExternalInput").ap()
    io["xv"] = nc.dram_tensor("xv", [N, C], FP, kind="ExternalInput").ap()
    io["gamma"] = nc.dram_tensor("gamma", [C], FP, kind="ExternalInput").ap()
    io["beta"] = nc.dram_tensor("beta", [C], FP, kind="ExternalInput").ap()
    io["Wq"] = nc.dram_tensor("Wq", [C, INNER], BF, kind="ExternalInput").ap()
    io["Wk"] = nc.dram_tensor("Wk", [C, INNER], BF, kind="ExternalInput").ap()
    io["Wv"] = nc.dram_tensor("Wv", [C, INNER], BF, kind="ExternalInput").ap()
    io["Wo"] = nc.dram_tensor("Wo", [INNER, C], BF, kind="ExternalInput").ap()
    io["bq"] = nc.dram_tensor("bq", [INNER], FP, kind="ExternalInput").ap()
    io["bk"] = nc.dram_tensor("bk", [INNER], FP, kind="ExternalInput").ap()
    io["bo"] = nc.dram_tensor("bo", [C], FP, kind="ExternalInput").ap()
    io["out"] = nc.dram_tensor("out", [NQ, C], FP, kind="ExternalOutput").ap()

    wsem = nc.alloc_semaphore("wsplit")
    with tile.TileContext(nc) as tc:
        _emit(nc, tc, io)
    _split_dma_waits(nc, wsem)
    _NC_CACHE = nc
    return nc


def make_in_maps(q, k, v, ln_g, ln_b, Wq, bq, Wk, bk, Wv, bv, Wo, bo):
    bf = ml_dtypes.bfloat16
    shared = {
        "gamma": np.ascontiguousarray(ln_g, np.float32),
        "beta": np.ascontiguousarray(ln_b, np.float32),
        "Wq": np.ascontiguousarray(Wq).astype(bf),
        "Wk": np.ascontiguousarray(Wk).astype(bf),
        "Wv": np.ascontiguousarray(Wv).astype(bf),
        "Wo": np.ascontiguousarray(Wo).astype(bf),
        "bq": np.ascontiguousarray(bq, np.float32),
        "bk": np.ascontiguousarray(bk, np.float32),
        "bo": (np.asarray(bo, np.float64)
               + np.asarray(bv, np.float64) @ np.asarray(Wo, np.float64)
               ).astype(np.float32),
    }
    in_maps = []
    for core in range(NCORES):
        b, h = core // 2, core % 2
        m = dict(shared)
        m["xq"] = np.ascontiguousarray(q[b, h * NQ:(h + 1) * NQ, :], np.float32)
        m["xk"] = np.ascontiguousarray(k[b], np.float32)
        m["xv"] = np.ascontiguousarray(v[b], np.float32)
        in_maps.append(m)
    return in_maps


def kernel(q, k, v, ln_g, ln_b, Wq, bq, Wk, bk, Wv, bv, Wo, bo, **run_kwargs):
    nc = build_nc()
    in_maps = make_in_maps(q, k, v, ln_g, ln_b, Wq, bq, Wk, bk, Wv, bv, Wo, bo)
    res = run_bass_kernel_spmd(nc, in_maps, core_ids=list(range(NCORES)),
                               **run_kwargs)
    out = np.empty((B, N, C), np.float32)
    for core in range(NCORES):
        b, h = core // 2, core % 2
        out[b, h * NQ:(h + 1) * NQ, :] = res.results[core]["out"]
    if run_kwargs:
        kernel.last_results = res
    return out


# revision 44
# speedup vs baseline: 5.1408x; 5.1408x over previous
"""Trainium2 Bass kernel for nn_Attention (dense transformer block, full-dim attention).

Reference computation (per batch b):
    qn/kn/vn = LayerNorm(q/k/v[b])           # over C=256
    qp = qn @ Wq + bq                        # [N, INNER]
    kp = kn @ Wk + bk
    vp = vn @ Wv + bv
    S  = qp @ kp.T * INNER_HEAD^-0.5         # [N, N]
    P  = softmax(S, axis=-1)
    out= (P @ vp) @ Wo + bo                  # [N, C]

Sharding: 8 cores = 4 batches x 2 query-row halves. Each core handles one
batch's full k/v (duplicated LN+projection within the pair -- no collectives)
and 1024 of the 2048 query rows.

On-chip dataflow (per core), all matmul contractions on the partition dim:
    xnT      : LN in natural layout ([tok,C] tiles), DMA-transpose to [C, tok]
    qpT, kpT : [INNER, tok] = Wq/Wk chunks (lhsT) x xnT (rhs)
    vp       : [tok, INNER] = vnT tiles (lhsT) x Wv (rhs)
    S^T      : [ktok, qtok] = kpT tiles (lhsT) x qpT (rhs), PSUM f32
    expS^T   : ACT exp(0.125 * S^T) straight out of PSUM, bf16
    rowsum   : ones-vector matmuls over expS^T (accumulated in PSUM)
    X~T      : [INNER, qtok] = vp tiles (lhsT) x expS^T (rhs)  [unnormalized]
    out      : [qtok, C] = X~T tiles (lhsT) x Wo (rhs); epilogue divides by
               rowsum (softmax normalization commutes with the row-linear
               output projection) and adds bo.
"""

import numpy as np
import ml_dtypes

import concourse.bass as bass
import concourse.tile as tile
from concourse import mybir
from concourse.bass_utils import run_bass_kernel_spmd

# Problem shapes (hardcoded per contract)
B = 4
N = 2048          # sequence length (k/v tokens per core)
C = 256           # channels
INNER = 1024      # inner projection dim
NQ = 1024         # query rows per core (N/2)
EPS = 1e-5
SCALE = 0.125     # 64 ** -0.5
P = 128

FP = mybir.dt.float32
BF = mybir.dt.bfloat16

NCORES = 8
CCH = C // P          # 2 chunks of the channel dim
JT = INNER // P       # 8 tiles of the inner dim
MT = N // P           # 16 k-token tiles
QT = NQ // P          # 8 q-token tiles
QCH = NQ // 512       # 2 q-token free chunks
KCH = N // 512        # 4 k-token free chunks

_sub = mybir.AluOpType.subtract
_mult = mybir.AluOpType.mult


def _bcast(ap, parts=P):
    # prepend a stride-0 partition dim: [n] -> [parts, n]
    return bass.AP(tensor=ap.tensor, offset=ap.offset,
                   ap=[[0, parts]] + [list(d) for d in ap.ap])


def _emit(nc, tc, io):
    from contextlib import ExitStack

    with ExitStack() as ctx:
        consts = ctx.enter_context(tc.tile_pool(name="consts", bufs=1))
        big = ctx.enter_context(tc.tile_pool(name="big", bufs=1))
        ln_pool = ctx.enter_context(tc.tile_pool(name="ln", bufs=4))
        lnx_pool = ctx.enter_context(tc.tile_pool(name="lnx", bufs=12))
        stat = ctx.enter_context(tc.tile_pool(name="stat", bufs=4))
        temps = ctx.enter_context(tc.tile_pool(name="temps", bufs=3))
        psum = ctx.enter_context(tc.tile_pool(name="psum", bufs=5, space="PSUM"))
        psum_rs = ctx.enter_context(tc.tile_pool(name="psum_rs", bufs=1, space="PSUM"))
        psum_t = ctx.enter_context(tc.tile_pool(name="psum_t", bufs=2, space="PSUM"))

        # ---- constants ------------------------------------------------
        # gamma/beta live on partitions in the transposed layout: [128, CCH]
        gT_sb = consts.tile([P, CCH], FP)
        nc.sync.dma_start(gT_sb, io["gamma"].rearrange("(c p) -> p c", p=P))
        bT_sb = consts.tile([P, CCH], FP)
        nc.sync.dma_start(bT_sb, io["beta"].rearrange("(c p) -> p c", p=P))
        bo_b = consts.tile([P, C], FP)
        nc.gpsimd.dma_start(bo_b, _bcast(io["bo"]))

        bq_sb = consts.tile([P, JT], FP)
        nc.scalar.dma_start(bq_sb, io["bq"].rearrange("(j p) -> p j", p=P))
        bk_sb = consts.tile([P, JT], FP)
        nc.scalar.dma_start(bk_sb, io["bk"].rearrange("(j p) -> p j", p=P))

        wq_sb = consts.tile([P, CCH, INNER], BF)
        nc.scalar.dma_start(wq_sb, io["Wq"].rearrange("(c p) n -> p c n", p=P))
        wk_sb = consts.tile([P, CCH, INNER], BF)
        nc.scalar.dma_start(wk_sb, io["Wk"].rearrange("(c p) n -> p c n", p=P))
        wv_sb = consts.tile([P, CCH, INNER], BF)
        nc.scalar.dma_start(wv_sb, io["Wv"].rearrange("(c p) n -> p c n", p=P))
        wo_sb = consts.tile([P, JT, C], BF)
        nc.scalar.dma_start(wo_sb, io["Wo"].rearrange("(j p) n -> p j n", p=P))

        ones_sb = consts.tile([P, 1], BF)
        nc.vector.memset(ones_sb, 1.0)
        eps_sb = consts.tile([P, 1], FP)
        nc.vector.memset(eps_sb, EPS)
        ident = consts.tile([P, P], BF)
        from concourse.masks import make_identity
        make_identity(nc, ident)

        # ---- persistent activations ----------------------------------
        qnT = big.tile([P, CCH, NQ], BF)
        knT = big.tile([P, CCH, N], BF)
        vnT = big.tile([P, CCH, N], BF)
        qpT = big.tile([P, JT, NQ], BF)
        kpT = big.tile([P, JT, N], BF)
        vp = big.tile([P, MT, INNER], BF)
        expS = big.tile([P, MT, NQ], BF)
        xT = big.tile([P, JT, NQ], BF)
        recip = big.tile([P, QT], FP)

        # PE warm-up during the LN-chain startup bubble: sustained activity
        # releases the HAM clock gate (1.2 -> 2.4 GHz) before real matmuls
        warm = psum_t.tile([P, P], BF, tag="pst", name="warm")
        for w in range(150):
            nc.tensor.transpose(warm, ident, ident)

        # ---- phase 1: layernorm + transpose --------------------------
        def layernorm(x_dram, ntiles, dstT):
            # groups of 8 tiles: batched stats -> one sqrt/recip per group,
            # then apply + transpose via PE (idle during this phase)
            for g0 in range(0, ntiles, 8):
                gn = min(8, ntiles - g0)
                mv_g = stat.tile([P, 8, 2], FP, tag="mv_g")
                xts = []
                for ii in range(gn):
                    i = g0 + ii
                    xt = lnx_pool.tile([P, C], FP, tag="xt")
                    nc.sync.dma_start(xt, x_dram[i * P:(i + 1) * P, :])
                    st = stat.tile([P, 6], FP, tag="st")
                    nc.vector.bn_stats(st, xt)
                    nc.vector.bn_aggr(mv_g[:, ii, :], st)
                    xts.append(xt)
                rstd_g = stat.tile([P, 8], FP, tag="rstd_g")
                nc.scalar.activation(rstd_g[:, :gn], mv_g[:, :gn, 1],
                                     mybir.ActivationFunctionType.Sqrt,
                                     bias=eps_sb, scale=1.0)
                nc.vector.reciprocal(rstd_g[:, :gn], rstd_g[:, :gn])
                for ii in range(gn):
                    i = g0 + ii
                    xn = ln_pool.tile([P, C], BF, tag="xn")
                    nc.vector.tensor_scalar(xn, xts[ii], mv_g[:, ii, 0:1],
                                            rstd_g[:, ii:ii + 1],
                                            op0=_sub, op1=_mult)
                    for c in range(CCH):
                        pst = psum_t.tile([P, P], BF, tag="pst")
                        nc.tensor.transpose(pst, xn[:, c * P:(c + 1) * P],
                                            ident)
                        # gamma/beta fused into the PSUM eviction: in the
                        # transposed layout they are per-partition scalars
                        dst = dstT[:, c, i * P:(i + 1) * P]
                        if (i + c) % 2 == 0:
                            nc.vector.tensor_scalar(dst, pst,
                                                    gT_sb[:, c:c + 1],
                                                    bT_sb[:, c:c + 1],
                                                    op0=_mult,
                                                    op1=mybir.AluOpType.add)
                        else:
                            nc.scalar.activation(
                                dst, pst,
                                mybir.ActivationFunctionType.Identity,
                                bias=bT_sb[:, c:c + 1],
                                scale=gT_sb[:, c:c + 1])

        # ---- phase 2: projections (interleaved per tensor with LN) ---
        def proj_T(srcT, w_sb, b_sb, dst, nch, evict_act):
            # dst[P(inner j), j, tok] = (x @ W).T + b   per inner tile j
            for j in range(JT):
                for n in range(nch):
                    ps = psum.tile([P, 512], FP, tag="ps")
                    for c in range(CCH):
                        nc.tensor.matmul(ps,
                                         lhsT=w_sb[:, c, j * P:(j + 1) * P],
                                         rhs=srcT[:, c, n * 512:(n + 1) * 512],
                                         start=(c == 0), stop=(c == CCH - 1))
                    d = dst[:, j, n * 512:(n + 1) * 512]
                    if evict_act and (j + n) % 2 == 0:
                        nc.scalar.activation(
                            d, ps, mybir.ActivationFunctionType.Identity,
                            bias=b_sb[:, j:j + 1], scale=1.0)
                    else:
                        nc.vector.tensor_scalar_add(d, ps, b_sb[:, j:j + 1])

        layernorm(io["xq"], QT, qnT)
        proj_T(qnT, wq_sb, bq_sb, qpT, QCH, evict_act=True)
        layernorm(io["xk"], MT, knT)
        proj_T(knT, wk_sb, bk_sb, kpT, KCH, evict_act=True)

        layernorm(io["xv"], MT, vnT)

        # vp natural: [tok(m), INNER]
        for m in range(MT):
            for jc in range(2):
                ps = psum.tile([P, 512], FP, tag="ps")
                for c in range(CCH):
                    nc.tensor.matmul(ps,
                                     lhsT=vnT[:, c, m * P:(m + 1) * P],
                                     rhs=wv_sb[:, c, jc * 512:(jc + 1) * 512],
                                     start=(c == 0), stop=(c == CCH - 1))
                d = vp[:, m, jc * 512:(jc + 1) * 512]
                if (m + jc) % 2 == 0:
                    nc.vector.tensor_copy(d, ps)
                else:
                    nc.scalar.copy(d, ps)

        # ---- phase 3: S^T = kp @ qp.T, exp ---------------------------
        # (after LN(v): all ACT Sqrt ops precede all Exp ops -- Sqrt and Exp
        # live in different activation table sets, each switch costs ~2.7us)
        for m in range(MT):
            for n in range(QCH):
                ps = psum.tile([P, 512], FP, tag="ps")
                for j in range(JT):
                    nc.tensor.matmul(ps,
                                     lhsT=kpT[:, j, m * P:(m + 1) * P],
                                     rhs=qpT[:, j, n * 512:(n + 1) * 512],
                                     start=(j == 0), stop=(j == JT - 1))
                nc.scalar.activation(expS[:, m, n * 512:(n + 1) * 512], ps,
                                     mybir.ActivationFunctionType.Exp,
                                     scale=SCALE)

        # rowsums: lhsT = expS tile, rhs = ones -> [128,1] per q-tile,
        # all 8 tiles sharing one PSUM bank (sequential accumulation groups)
        rs8 = psum_rs.tile([P, QT], FP, tag="rs8")
        for t in range(QT):
            for m in range(MT):
                nc.tensor.matmul(rs8[:, t:t + 1],
                                 lhsT=expS[:, m, t * P:(t + 1) * P],
                                 rhs=ones_sb,
                                 start=(m == 0), stop=(m == MT - 1))
        nc.vector.reciprocal(recip, rs8)

        # ---- phase 4: X~T = vp.T-tiles x expS^T ----------------------
        for n in range(QCH):
            for j in range(JT):
                ps = psum.tile([P, 512], FP, tag="ps")
                for m in range(MT):
                    nc.tensor.matmul(ps,
                                     lhsT=vp[:, m, j * P:(j + 1) * P],
                                     rhs=expS[:, m, n * 512:(n + 1) * 512],
                                     start=(m == 0), stop=(m == MT - 1))
                d = xT[:, j, n * 512:(n + 1) * 512]
                if (n + j) % 2 == 0:
                    nc.vector.tensor_copy(d, ps)
                else:
                    nc.scalar.copy(d, ps)

        # ---- phase 5: out = X~T.T @ Wo, normalize + bias -------------
        for t in range(QT):
            ps = psum.tile([P, 512], FP, tag="ps")
            po = ps[:, :C]
            for j in range(JT):
                nc.tensor.matmul(po,
                                 lhsT=xT[:, j, t * P:(t + 1) * P],
                                 rhs=wo_sb[:, j, :],
                                 start=(j == 0), stop=(j == JT - 1))
            o1 = temps.tile([P, C], FP, tag="o1")
            nc.vector.tensor_scalar(o1, po, recip[:, t:t + 1], None, op0=_mult)
            o2 = temps.tile([P, C], FP, tag="o2")
            nc.vector.tensor_add(o2, o1, bo_b)
            nc.sync.dma_start(io["out"][t * P:(t + 1) * P, :], o2)


_DMA_WAIT_LIMIT = 1
_ENGINE_WAIT_LIMIT = 1


def _split_dma_waits(nc, wsem):
    """This walrus's instruction structs carry very few sync-wait slots
    (DMA_DIRECT2D effectively 1, engine ops ~2); Tile can emit more. Move the
    excess onto an EventSemaphore wait on the issuing engine right before the
    instruction (engine streams are in-order, so this is a conservative,
    correct strengthening)."""
    import bass_rust
    fn = nc.m.functions[0]
    for blk in fn.blocks:
        il = list(blk.instructions)
        out = []
        changed = False
        for inst in il:
            tn = type(inst).__name__
            si = inst.sync_info
            if si is not None and tn != "InstEventSemaphore":
                limit = _DMA_WAIT_LIMIT if ("DMA" in tn or "Dma" in tn) \
                    else _ENGINE_WAIT_LIMIT
                w = list(si.on_wait)
                if len(w) > limit:
                    excess = w[:-limit]
                    # EventSemaphore carries <=2 waits and <=1 update; chain
                    # as many as needed, each ticking the dummy wsplit sem.
                    for gi in range(0, len(excess), 2):
                        nop = mybir.InstEventSemaphore(
                            name=f"wsplit{gi}_{inst.name}", ins=[], outs=[])
                        nop.engine = inst.engine
                        nop.sync_info = bass_rust.SyncInfo(
                            on_wait=excess[gi:gi + 2],
                            on_update=[bass_rust.SyncUpdate(
                                sync_type="semaphore", id=wsem.num,
                                ant_name=wsem.name, update_mode="sem-add-imm",
                                update_value=1)])
                        out.append(nop)
                    si.on_wait = w[-limit:]
                    changed = True
            out.append(inst)
        if changed:
            blk.instructions = out


_NC_CACHE = {}


def build_nc(reps=1):
    global _NC_CACHE
    if reps in _NC_CACHE:
        return _NC_CACHE[reps]
    nc = bass.Bass("TRN2", target_bir_lowering=False, debug=False,
                   num_devices=NCORES)
    io = {}
    io["xq"] = nc.dram_tensor("xq", [NQ, C], FP, kind="ExternalInput").ap()
    io["xk"] = nc.dram_tensor("xk", [N, C], FP, kind="ExternalInput").ap()
    io["xv"] = nc.dram_tensor("xv", [N, C], FP, kind="ExternalInput").ap()
    io["gamma"] = nc.dram_tensor("gamma", [C], FP, kind="ExternalInput").ap()
    io["beta"] = nc.dram_tensor("beta", [C], FP, kind="ExternalInput").ap()
    io["Wq"] = nc.dram_tensor("Wq", [C, INNER], BF, kind="ExternalInput").ap()
    io["Wk"] = nc.dram_tensor("Wk", [C, INNER], BF, kind="ExternalInput").ap()
    io["Wv"] = nc.dram_tensor("Wv", [C, INNER], BF, kind="ExternalInput").ap()
    io["Wo"] = nc.dram_tensor("Wo", [INNER, C], BF, kind="ExternalInput").ap()
    io["bq"] = nc.dram_tensor("bq", [INNER], FP, kind="ExternalInput").ap()
    io["bk"] = nc.dram_tensor("bk", [INNER], FP, kind="ExternalInput").ap()
    io["bo"] = nc.dram_tensor("bo", [C], FP, kind="ExternalInput").ap()
    io["out"] = nc.dram_tensor("out", [NQ, C], FP, kind="ExternalOutput").ap()

    wsem = nc.alloc_semaphore("wsplit")
    with tile.TileContext(nc) as tc:
        for _ in range(reps):
            _emit(nc, tc, io)
    _split_dma_waits(nc, wsem)
    _NC_CACHE[reps] = nc
    return nc


def make_in_maps(q, k, v, ln_g, ln_b, Wq, bq, Wk, bk, Wv, bv, Wo, bo):
    bf = ml_dtypes.bfloat16
    shared = {
        "gamma": np.ascontiguousarray(ln_g, np.float32),
        "beta": np.ascontiguousarray(ln_b, np.float32),
        "Wq": np.ascontiguousarray(Wq).astype(bf),
        "Wk": np.ascontiguousarray(Wk).astype(bf),
        "Wv": np.ascontiguousarray(Wv).astype(bf),
        "Wo": np.ascontiguousarray(Wo).astype(bf),
        "bq": np.ascontiguousarray(bq, np.float32),
        "bk": np.ascontiguousarray(bk, np.float32),
        "bo": (np.asarray(bo, np.float64)
               + np.asarray(bv, np.float64) @ np.asarray(Wo, np.float64)
               ).astype(np.float32),
    }
    in_maps = []
    for core in range(NCORES):
        b, h = core // 2, core % 2
        m = dict(shared)
        m["xq"] = np.ascontiguousarray(q[b, h * NQ:(h + 1) * NQ, :], np.float32)
        m["xk"] = np.ascontiguousarray(k[b], np.float32)
        m["xv"] = np.ascontiguousarray(v[b], np.float32)
        in_maps.append(m)
    return in_maps


def kernel(q, k, v, ln_g, ln_b, Wq, bq, Wk, bk, Wv, bv, Wo, bo, **run_kwargs):
    nc = build_nc()
    in_maps = make_in_maps(q, k, v, ln_g, ln_b, Wq, bq, Wk, bk, Wv, bv, Wo, bo)
    try:
        res = run_bass_kernel_spmd(nc, in_maps, core_ids=list(range(NCORES)),
                                   **run_kwargs)
    except Exception:
        # transient axon-tunnel failures happen; one retry
        res = run_bass_kernel_spmd(nc, in_maps, core_ids=list(range(NCORES)),
                                   **run_kwargs)
    out = np.empty((B, N, C), np.float32)
    for core in range(NCORES):
        b, h = core // 2, core % 2
        out[b, h * NQ:(h + 1) * NQ, :] = res.results[core]["out"]
    if run_kwargs:
        kernel.last_results = res
    return out
